# revision 1
# baseline (speedup 1.0000x reference)
"""Trainium2 Bass kernel for a LoRA-MoE layer (gate top-2 softmax routing +
dense base linear + per-expert low-rank adapters), SPMD across 8 NeuronCores.

Math (per token t):
    logits = x @ gate_w.T                      # [E]
    top-2 softmax over logits -> dense w[E] (0 for non-selected)
    out = x @ base_w.T + base_b
        + SCALING * sum_e w[e] * (x @ lora_A[e].T) @ lora_B[e].T

Key identity used: with w folded into the rank-space activations,
    lora_out = (low * w_rep) @ B_all.T,  low = x @ A_all.T   (A_all: [E*R, D])
so the whole MoE-LoRA is two dense matmuls + tiny gating vector math.

Sharding: 4-way over tokens x 2-way over out-features (8 cores, no
collectives).  Per core: T=1024 tokens, TO=2048 out features.

Layout per core (everything "transposed", contraction dim on partitions):
    out.T[o, t] = sum_d W[o, d] * x.T[d, t]    (x.T moving, W tiles stationary)
"""

import numpy as np

import concourse.bass as bass
import concourse.bass_isa as bass_isa
import concourse.mybir as mybir
import concourse.tile as tile
from concourse import bacc
from concourse.bass_utils import run_bass_kernel_spmd

F32 = mybir.dt.float32
F32R = mybir.dt.float32r

# Problem constants
B, S, D, O = 2, 2048, 4096, 4096
E, R = 8, 16
ER = E * R  # 128
SCALING = 32.0 / 16.0

# Sharding: 4 token groups x 2 out-feature groups
N_CORES = 8
TG, OG = 4, 2
T = (B * S) // TG       # 1024 tokens per core
TO = O // OG            # 2048 out features per core
KT = D // 128           # 32 contraction tiles
OTN = TO // 128         # 16 out tiles per core
CH = 2                  # token chunks of 512
CHW = T // CH           # 512


def build_body(nc, tc, tensors, mm_dt=F32R, gate_dt=F32R):
    xT, wT, aT, gT, bT, bias2, Rm, out = tensors
    AX_C = mybir.AxisListType.C
    OP = mybir.AluOpType

    with (
        tc.tile_pool(name="xp", bufs=KT) as xp,
        tc.tile_pool(name="wp", bufs=4) as wp,
        tc.tile_pool(name="cst", bufs=1) as cst,
        tc.tile_pool(name="apl", bufs=3) as apl,
        tc.tile_pool(name="gw", bufs=1) as gw,
        tc.tile_pool(name="outp", bufs=2) as outp,
        tc.tile_pool(name="psA", bufs=2, space="PSUM") as psA,
        tc.tile_pool(name="psB", bufs=4, space="PSUM") as psB,
    ):
        # ---- constants ----
        bT_sb = cst.tile([ER, TO], F32R)
        nc.gpsimd.dma_start(out=bT_sb[:], in_=bT[:].bitcast(F32R))
        bias_sb = cst.tile([128, OTN], F32)
        nc.gpsimd.dma_start(out=bias_sb[:], in_=bias2[:])
        Rm_sb = cst.tile([E, ER], F32R)
        nc.gpsimd.dma_start(out=Rm_sb[:], in_=Rm[:].bitcast(F32R))

        # ---- resident x.T tiles ----
        x_tiles = []
        for k in range(KT):
            xk = xp.tile([128, T], F32R, tag="x", name=f"x{k}")
            nc.sync.dma_start(out=xk[:], in_=xT[:, k, :].bitcast(F32R))
            x_tiles.append(xk)

        # ---- phase A: low.T = A_all.T^T @ x.T ; gate.T = g^T @ x.T ----
        low_ps = [psA.tile([ER, CHW], F32, tag="low", name=f"lowps{c}") for c in range(CH)]
        gate_ps = [psA.tile([E, CHW], F32, tag="gate", name=f"gateps{c}") for c in range(CH)]
        for k in range(KT):
            ak = apl.tile([128, ER], F32R, tag="a", name=f"a{k}")
            nc.gpsimd.dma_start(out=ak[:], in_=aT[:, k, :].bitcast(F32R))
            gk = apl.tile([128, E], F32R, tag="g", name=f"g{k}")
            nc.gpsimd.dma_start(out=gk[:], in_=gT[:, k, :].bitcast(F32R))
            for c in range(CH):
                rhs = x_tiles[k][:, c * CHW:(c + 1) * CHW]
                nc.tensor.matmul(low_ps[c][:], lhsT=ak[:], rhs=rhs,
                                 start=(k == 0), stop=(k == KT - 1))
                nc.tensor.matmul(gate_ps[c][:], lhsT=gk[:], rhs=rhs,
                                 start=(k == 0), stop=(k == KT - 1))

        # ---- gating math in [E, t] layout, per 512-token chunk ----
        lowT_sb = gw.tile([ER, T], F32R, tag="lowT")
        for c in range(CH):
            cs = slice(c * CHW, (c + 1) * CHW)
            g_sb = gw.tile([E, CHW], F32, tag="gsb", name=f"gsb{c}")
            nc.vector.tensor_copy(g_sb[:], gate_ps[c][:])
            m1b = gw.tile([E, CHW], F32, tag="m1b", name=f"m1b{c}")
            nc.gpsimd.partition_all_reduce(m1b[:], g_sb[:], channels=E,
                                           reduce_op=bass_isa.ReduceOp.max)
            eq = gw.tile([E, CHW], F32, tag="tmp", bufs=3, name=f"eq{c}")
            nc.vector.tensor_tensor(eq[:], g_sb[:], m1b[:], op=OP.is_equal)
            gm = gw.tile([E, CHW], F32, tag="tmp", bufs=3, name=f"gm{c}")
            nc.vector.scalar_tensor_tensor(gm[:], in0=eq[:], scalar=-1e30, in1=g_sb[:],
                                           op0=OP.mult, op1=OP.add)
            m2b = gw.tile([E, CHW], F32, tag="m2b", name=f"m2b{c}")
            nc.gpsimd.partition_all_reduce(m2b[:], gm[:], channels=E,
                                           reduce_op=bass_isa.ReduceOp.max)
            diff = gw.tile([E, CHW], F32, tag="tmp", bufs=3, name=f"diff{c}")
            nc.vector.tensor_sub(diff[:], g_sb[:], m1b[:])
            ex = gw.tile([E, CHW], F32, tag="ex", name=f"ex{c}")
            nc.scalar.activation(ex[:], diff[:], mybir.ActivationFunctionType.Exp)
            mask = gw.tile([E, CHW], F32, tag="tmp", bufs=3, name=f"mask{c}")
            nc.vector.tensor_tensor(mask[:], g_sb[:], m2b[:], op=OP.is_ge)
            wn = gw.tile([E, CHW], F32, tag="wn", name=f"wn{c}")
            nc.vector.tensor_mul(wn[:], ex[:], mask[:])
            # denominator 1 + exp(m2 - m1), computed broadcast on all 8 rows
            dmb = gw.tile([E, CHW], F32, tag="tmp", bufs=3, name=f"dmb{c}")
            nc.vector.tensor_sub(dmb[:], m2b[:], m1b[:])
            edb = gw.tile([E, CHW], F32, tag="edb", name=f"edb{c}")
            nc.scalar.activation(edb[:], dmb[:], mybir.ActivationFunctionType.Exp)
            denb = gw.tile([E, CHW], F32, tag="tmp", bufs=3, name=f"denb{c}")
            nc.vector.tensor_scalar_add(denb[:], edb[:], 1.0)
            recb = gw.tile([E, CHW], F32, tag="recb", name=f"recb{c}")
            nc.vector.reciprocal(recb[:], denb[:])
            wsc = gw.tile([E, CHW], F32R, tag="wsc", name=f"wsc{c}")
            nc.vector.scalar_tensor_tensor(wsc[:], in0=wn[:], scalar=SCALING, in1=recb[:],
                                           op0=OP.mult, op1=OP.mult)
            # replicate each expert weight over its 16 ranks via tiny matmul
            wrep_ps = psA.tile([ER, CHW], F32, tag="gate", name=f"wrep{c}")
            nc.tensor.matmul(wrep_ps[:], lhsT=Rm_sb[:], rhs=wsc[:],
                             start=True, stop=True)
            # low_w.T = low.T * w_rep  (copy wrep to SBUF first: DVE has a
            # single PSUM read port, two-PSUM-operand tensor_tensor is illegal)
            wrep_sb = gw.tile([ER, CHW], F32, tag="wrepsb", name=f"wrepsb{c}")
            nc.scalar.copy(wrep_sb[:], wrep_ps[:])
            nc.vector.tensor_tensor(lowT_sb[:, cs], low_ps[c][:], wrep_sb[:], op=OP.mult)

        # ---- phase B: out.T tiles = W-tile^T @ x.T  (+ B-tile^T @ low_w.T) ----
        for ot in range(OTN):
            wtiles = []
            for q in range(4):
                wq = wp.tile([128, 8, 128], F32R, tag="w", name=f"w{ot}_{q}")
                nc.scalar.dma_start(out=wq[:], in_=wT[:, ot, q * 8:(q + 1) * 8, :].bitcast(F32R))
                wtiles.append(wq)
            pb = [psB.tile([128, CHW], F32, tag="pb", name=f"pb{ot}_{c}") for c in range(CH)]
            for k in range(KT):
                wk = wtiles[k // 8][:, k % 8, :]
                for c in range(CH):
                    nc.tensor.matmul(pb[c][:], lhsT=wk,
                                     rhs=x_tiles[k][:, c * CHW:(c + 1) * CHW],
                                     start=(k == 0), stop=False)
            for c in range(CH):
                nc.tensor.matmul(pb[c][:], lhsT=bT_sb[:, ot * 128:(ot + 1) * 128],
                                 rhs=lowT_sb[:, c * CHW:(c + 1) * CHW],
                                 start=False, stop=True)
            o_sb = outp.tile([128, T], F32, tag="o", name=f"o{ot}")
            for c in range(CH):
                nc.vector.tensor_scalar(o_sb[:, c * CHW:(c + 1) * CHW], pb[c][:],
                                        scalar1=bias_sb[:, ot:ot + 1], scalar2=None,
                                        op0=OP.add)
            nc.gpsimd.dma_start(out=out[:, ot, :], in_=o_sb[:])


def build_module(mm_dt=F32R, gate_dt=F32R, debug=False):
    nc = bacc.Bacc("TRN2", target_bir_lowering=False, debug=debug)
    xT = nc.dram_tensor("xT", [128, KT, T], F32, kind="ExternalInput")
    wT = nc.dram_tensor("wT", [128, OTN, KT, 128], F32, kind="ExternalInput")
    aT = nc.dram_tensor("aT", [128, KT, ER], F32, kind="ExternalInput")
    gT = nc.dram_tensor("gT", [128, KT, E], F32, kind="ExternalInput")
    bT = nc.dram_tensor("bT", [ER, TO], F32, kind="ExternalInput")
    bias2 = nc.dram_tensor("bias2", [128, OTN], F32, kind="ExternalInput")
    Rm = nc.dram_tensor("Rm", [E, ER], F32, kind="ExternalInput")
    out = nc.dram_tensor("out", [128, OTN, T], F32, kind="ExternalOutput")
    with tile.TileContext(nc) as tc:
        build_body(nc, tc, (xT, wT, aT, gT, bT, bias2, Rm, out),
                   mm_dt=mm_dt, gate_dt=gate_dt)
    nc.compile()
    return nc


def shard_inputs(x, gate_w, base_w, base_b, lora_A, lora_B):
    """FULL inputs -> list of 8 per-core input maps (host-side, free)."""
    x = np.asarray(x, dtype=np.float32)
    gate_w = np.asarray(gate_w, dtype=np.float32)
    base_w = np.asarray(base_w, dtype=np.float32)
    base_b = np.asarray(base_b, dtype=np.float32)
    lora_A = np.asarray(lora_A, dtype=np.float32)
    lora_B = np.asarray(lora_B, dtype=np.float32)

    xf = x.reshape(B * S, D)
    # replicated smalls
    gT = np.ascontiguousarray(gate_w.T.reshape(KT, 128, E).transpose(1, 0, 2))
    A_flat = lora_A.reshape(ER, D)
    aT = np.ascontiguousarray(A_flat.T.reshape(KT, 128, ER).transpose(1, 0, 2))
    B_flat = lora_B.transpose(0, 2, 1).reshape(ER, O)   # [er, o]
    Rm = np.repeat(np.eye(E, dtype=np.float32), R, axis=1)  # [E, ER]

    in_maps = []
    for c in range(N_CORES):
        tg, og = c // OG, c % OG
        x_c = xf[tg * T:(tg + 1) * T]                       # [T, D]
        xT = np.ascontiguousarray(x_c.T.reshape(KT, 128, T).transpose(1, 0, 2))
        w_c = base_w[og * TO:(og + 1) * TO]                 # [TO, D]
        wT = np.ascontiguousarray(
            w_c.reshape(OTN, 128, KT, 128).transpose(3, 0, 2, 1))
        bT = np.ascontiguousarray(B_flat[:, og * TO:(og + 1) * TO])
        bias2 = np.ascontiguousarray(base_b[og * TO:(og + 1) * TO].reshape(OTN, 128).T)
        in_maps.append({"xT": xT, "wT": wT, "aT": aT, "gT": gT,
                        "bT": bT, "bias2": bias2, "Rm": Rm})
    return in_maps


def gather_outputs(results):
    """list of 8 per-core result maps -> FULL output [B, S, O]."""
    full = np.empty((B * S, O), dtype=np.float32)
    for c in range(N_CORES):
        tg, og = c // OG, c % OG
        oc = results[c]["out"]                              # [128, OTN, T]
        full[tg * T:(tg + 1) * T, og * TO:(og + 1) * TO] = \
            oc.transpose(2, 1, 0).reshape(T, TO)
    return full.reshape(B, S, O)


_NC_CACHE = {}


def _get_module(mm_dt=F32R, gate_dt=F32R):
    key = (mm_dt, gate_dt)
    if key not in _NC_CACHE:
        _NC_CACHE[key] = build_module(mm_dt=mm_dt, gate_dt=gate_dt)
    return _NC_CACHE[key]


def run_sharded(in_maps, mm_dt=F32R, gate_dt=F32R, **run_kwargs):
    nc = _get_module(mm_dt=mm_dt, gate_dt=gate_dt)
    return run_bass_kernel_spmd(nc, in_maps, list(range(N_CORES)), **run_kwargs)


def kernel(x, gate_w, base_w, base_b, lora_A, lora_B):
    in_maps = shard_inputs(x, gate_w, base_w, base_b, lora_A, lora_B)
    res = run_sharded(in_maps)
    return gather_outputs(res.results)



# revision 2
# speedup vs baseline: 1.2035x; 1.2035x over previous
"""Trainium2 Bass kernel for a LoRA-MoE layer (gate top-2 softmax routing +
dense base linear + per-expert low-rank adapters), SPMD across 8 NeuronCores.

Math (per token t):
    logits = x @ gate_w.T                      # [E]
    top-2 softmax over logits -> dense w[E] (0 for non-selected)
    out = x @ base_w.T + base_b
        + SCALING * sum_e w[e] * (x @ lora_A[e].T) @ lora_B[e].T

Key identities:
  * w folded into rank-space activations: lora_out = (low * w_rep) @ B_all.T
    with low = x @ A_all.T (A_all: [E*R, D]) -> whole MoE-LoRA is two dense
    matmuls + tiny gating vector math.
  * top-2 softmax via sigmoid: w_top1 = sigmoid(l1-l2), w_top2 = sigmoid(l2-l1),
    i.e. w_e = [l_e >= m2] * sigmoid(2*l_e - m1 - m2).

Sharding: 4-way over tokens x 2-way over out-features (8 cores, no
collectives).  Per core: T=1024 tokens, TO=2048 out features.

Performance structure (vs the f32r baseline at ~360us):
  * all matmul operands cast to bf16 on the host (free) -> x loads in ~25us
    instead of 96us; PE matmul rate is identical (1 col/cycle).
  * DMA priority: x + adapters stream first on two queues; W tiles queue
    strictly behind x on the same rings so phase A is never starved.
  * phase A (low-rank + gate MMs) streams against x chunk arrival; phase B
    (base W MMs) follows with W-tile k-loops; the per-out-tile "stop" matmul
    (B-adapter fold-in) for ot=0 is deferred past ot=1's W MMs so the gating
    vector-math latency hides entirely behind PE work.
"""

import numpy as np
import ml_dtypes

import concourse.bass as bass
import concourse.bass_isa as bass_isa
import concourse.mybir as mybir
import concourse.tile as tile
from concourse import bacc
from concourse.bass_utils import run_bass_kernel_spmd

F32 = mybir.dt.float32
BF16 = mybir.dt.bfloat16
NPBF16 = ml_dtypes.bfloat16

# Problem constants
B, S, D, O = 2, 2048, 4096, 4096
E, R = 8, 16
ER = E * R  # 128
SCALING = 32.0 / 16.0

# Sharding: 4 token groups x 2 out-feature groups
N_CORES = 8
TG, OG = 4, 2
T = (B * S) // TG       # 1024 tokens per core
TO = O // OG            # 2048 out features per core
KT = D // 128           # 32 contraction tiles
OTN = TO // 128         # 16 out tiles per core
CH = 2                  # token chunks of 512 (PSUM bank width)
CHW = T // CH           # 512
XC = 8                  # x DMA chunks
KPC = KT // XC          # 4 k-tiles per x chunk


def build_body(nc, tc, tensors):
    xT, wT, aT, gT, bT, bias2, Rm, out = tensors
    OP = mybir.AluOpType
    ACT = mybir.ActivationFunctionType

    with (
        tc.tile_pool(name="xp", bufs=XC) as xp,
        tc.tile_pool(name="wp", bufs=8) as wp,
        tc.tile_pool(name="cst", bufs=1) as cst,
        tc.tile_pool(name="gw", bufs=1) as gw,
        tc.tile_pool(name="outp", bufs=2) as outp,
        tc.tile_pool(name="psA", bufs=2, space="PSUM") as psA,
        tc.tile_pool(name="psB", bufs=4, space="PSUM") as psB,
    ):
        # ---- DMA program.  scalar ring: smalls + odd x chunks + bT;
        #      sync ring: a, even x chunks, then all W (strictly after x). ----
        a_all = cst.tile([128, KT * ER], BF16)
        nc.sync.dma_start(out=a_all[:], in_=aT[:])
        g_all = cst.tile([128, KT * E], BF16)
        nc.scalar.dma_start(out=g_all[:], in_=gT[:])
        bias_sb = cst.tile([128, OTN], F32)
        nc.scalar.dma_start(out=bias_sb[:], in_=bias2[:])
        Rm_sb = cst.tile([E, ER], BF16)
        nc.scalar.dma_start(out=Rm_sb[:], in_=Rm[:])

        x_tiles = []
        for c in range(XC):
            xc_t = xp.tile([128, KPC * T], BF16, tag="x", name=f"x{c}")
            eng = nc.sync if c % 2 == 0 else nc.scalar
            eng.dma_start(out=xc_t[:], in_=xT[:, c, :])
            x_tiles.append(xc_t)

        bT_sb = cst.tile([ER, TO], BF16)
        nc.scalar.dma_start(out=bT_sb[:], in_=bT[:])

        w_tiles = []
        for ot in range(OTN):
            wv = wp.tile([128, KT * 128], BF16, tag="w", name=f"w{ot}")
            nc.sync.dma_start(out=wv[:], in_=wT[:, ot, :])
            w_tiles.append(wv)

        def xs(k, c):
            """x.T slice [128, CHW] for k-tile k, token chunk c."""
            return x_tiles[k // KPC][:, (k % KPC) * T + c * CHW:
                                     (k % KPC) * T + (c + 1) * CHW]

        # ---- phase A: low.T = A_all.T^T @ x.T ; gate.T = g^T @ x.T ----
        low_ps = [psA.tile([ER, CHW], F32, tag="low", name=f"lowps{c}")
                  for c in range(CH)]
        gate_ps = [psA.tile([E, CHW], F32, tag="gate", name=f"gateps{c}")
                   for c in range(CH)]
        for k in range(KT):
            gk = g_all[:, k * E:(k + 1) * E]
            ak = a_all[:, k * ER:(k + 1) * ER]
            for c in range(CH):
                nc.tensor.matmul(gate_ps[c][:], lhsT=gk, rhs=xs(k, c),
                                 start=(k == 0), stop=(k == KT - 1))
            for c in range(CH):
                nc.tensor.matmul(low_ps[c][:], lhsT=ak, rhs=xs(k, c),
                                 start=(k == 0), stop=(k == KT - 1))

        # ---- gating math in [E, t] layout (DVE/ACT/GPSIMD; overlaps the
        #      first base-W matmul groups on the PE) ----
        # w_e = [l_e >= m2] * sigmoid(2*l_e - m1 - m2) * SCALING
        lowT_sb = cst.tile([ER, T], BF16, tag="lowT")
        wsc = []
        for c in range(CH):
            g_sb = gw.tile([E, CHW], F32, tag="g", bufs=2, name=f"g{c}")
            nc.scalar.copy(g_sb[:], gate_ps[c][:])
            m1 = gw.tile([E, CHW], F32, tag="m1", bufs=2, name=f"m1{c}")
            nc.gpsimd.partition_all_reduce(m1[:], g_sb[:], channels=E,
                                           reduce_op=bass_isa.ReduceOp.max)
            eq = gw.tile([E, CHW], F32, tag="tmp", bufs=6, name=f"eq{c}")
            nc.vector.tensor_tensor(eq[:], g_sb[:], m1[:], op=OP.is_equal)
            gm = gw.tile([E, CHW], F32, tag="tmp", bufs=6, name=f"gm{c}")
            nc.vector.scalar_tensor_tensor(gm[:], in0=eq[:], scalar=-1e30,
                                           in1=g_sb[:], op0=OP.mult, op1=OP.add)
            m2 = gw.tile([E, CHW], F32, tag="m2", bufs=2, name=f"m2{c}")
            nc.gpsimd.partition_all_reduce(m2[:], gm[:], channels=E,
                                           reduce_op=bass_isa.ReduceOp.max)
            t1 = gw.tile([E, CHW], F32, tag="tmp", bufs=6, name=f"t1{c}")
            nc.vector.tensor_tensor(t1[:], m1[:], m2[:], op=OP.add)
            s = gw.tile([E, CHW], F32, tag="tmp", bufs=6, name=f"s{c}")
            nc.vector.scalar_tensor_tensor(s[:], in0=g_sb[:], scalar=2.0,
                                           in1=t1[:], op0=OP.mult,
                                           op1=OP.subtract)
            sig = gw.tile([E, CHW], F32, tag="sig", bufs=2, name=f"sig{c}")
            nc.scalar.activation(sig[:], s[:], ACT.Sigmoid)
            mask = gw.tile([E, CHW], F32, tag="tmp", bufs=6, name=f"mask{c}")
            nc.vector.tensor_tensor(mask[:], g_sb[:], m2[:], op=OP.is_ge)
            wc = gw.tile([E, CHW], BF16, tag="wsc", bufs=2, name=f"wsc{c}")
            nc.vector.scalar_tensor_tensor(wc[:], in0=sig[:], scalar=SCALING,
                                           in1=mask[:], op0=OP.mult,
                                           op1=OP.mult)
            wsc.append(wc)

        # ---- phase B: out.T[ot] = sum_k W[ot,k]^T @ x.T (+ B^T @ low_w.T) ----
        def w_group(ot, pb):
            for k in range(KT):
                wk = w_tiles[ot][:, k * 128:(k + 1) * 128]
                for c in range(CH):
                    nc.tensor.matmul(pb[c][:], lhsT=wk, rhs=xs(k, c),
                                     start=(k == 0), stop=False)

        def stop_group(ot, pb):
            o_sb = outp.tile([128, T], F32, tag="o", name=f"o{ot}")
            for c in range(CH):
                nc.tensor.matmul(pb[c][:],
                                 lhsT=bT_sb[:, ot * 128:(ot + 1) * 128],
                                 rhs=lowT_sb[:, c * CHW:(c + 1) * CHW],
                                 start=False, stop=True)
            for c in range(CH):
                cs = slice(c * CHW, (c + 1) * CHW)
                nc.vector.tensor_scalar(o_sb[:, cs], pb[c][:],
                                        scalar1=bias_sb[:, ot:ot + 1],
                                        scalar2=None, op0=OP.add)
            nc.gpsimd.dma_start(out=out[:, ot, :], in_=o_sb[:])

        pbs = []
        for ot in range(OTN):
            pb = [psB.tile([128, CHW], F32, tag="pb", name=f"pb{ot}_{c}")
                  for c in range(CH)]
            pbs.append(pb)
            w_group(ot, pb)
            if ot == 0:
                # replicate each expert weight over its 16 ranks via a tiny
                # matmul, then fold into the rank-space activations.  Emitted
                # on the tensor queue after ot0's W MMs so the PE never waits
                # on the gating chain.
                for c in range(CH):
                    cs = slice(c * CHW, (c + 1) * CHW)
                    wrep_ps = psA.tile([ER, CHW], F32, tag="gate",
                                       name=f"wrep{c}")
                    nc.tensor.matmul(wrep_ps[:], lhsT=Rm_sb[:], rhs=wsc[c][:],
                                     start=True, stop=True)
                    wrep_sb = gw.tile([ER, CHW], F32, tag="wrepsb", bufs=2,
                                      name=f"wrepsb{c}")
                    nc.scalar.copy(wrep_sb[:], wrep_ps[:])
                    # low_w.T = low.T * w_rep (DVE: one PSUM operand only)
                    nc.vector.tensor_tensor(lowT_sb[:, cs], low_ps[c][:],
                                            wrep_sb[:], op=OP.mult)
            else:
                stop_group(ot - 1, pbs[ot - 1])
        stop_group(OTN - 1, pbs[OTN - 1])


def build_module(debug=False):
    nc = bacc.Bacc("TRN2", target_bir_lowering=False, debug=debug)
    xT = nc.dram_tensor("xT", [128, XC, KPC * T], BF16, kind="ExternalInput")
    wT = nc.dram_tensor("wT", [128, OTN, KT * 128], BF16, kind="ExternalInput")
    aT = nc.dram_tensor("aT", [128, KT * ER], BF16, kind="ExternalInput")
    gT = nc.dram_tensor("gT", [128, KT * E], BF16, kind="ExternalInput")
    bT = nc.dram_tensor("bT", [ER, TO], BF16, kind="ExternalInput")
    bias2 = nc.dram_tensor("bias2", [128, OTN], F32, kind="ExternalInput")
    Rm = nc.dram_tensor("Rm", [E, ER], BF16, kind="ExternalInput")
    out = nc.dram_tensor("out", [128, OTN, T], F32, kind="ExternalOutput")
    with tile.TileContext(nc) as tc:
        build_body(nc, tc, (xT, wT, aT, gT, bT, bias2, Rm, out))
    nc.compile()
    return nc


def shard_inputs(x, gate_w, base_w, base_b, lora_A, lora_B):
    """FULL inputs -> list of 8 per-core input maps (host-side, free)."""
    x = np.asarray(x, dtype=np.float32)
    gate_w = np.asarray(gate_w, dtype=np.float32)
    base_w = np.asarray(base_w, dtype=np.float32)
    base_b = np.asarray(base_b, dtype=np.float32)
    lora_A = np.asarray(lora_A, dtype=np.float32)
    lora_B = np.asarray(lora_B, dtype=np.float32)

    xf = x.reshape(B * S, D)
    # replicated smalls
    gT = np.ascontiguousarray(
        gate_w.T.reshape(KT, 128, E).transpose(1, 0, 2)
        .reshape(128, KT * E)).astype(NPBF16)
    A_flat = lora_A.reshape(ER, D)
    aT = np.ascontiguousarray(
        A_flat.T.reshape(KT, 128, ER).transpose(1, 0, 2)
        .reshape(128, KT * ER)).astype(NPBF16)
    B_flat = lora_B.transpose(0, 2, 1).reshape(ER, O)   # [er, o]
    Rm = np.repeat(np.eye(E, dtype=np.float32), R, axis=1).astype(NPBF16)

    in_maps = []
    for c in range(N_CORES):
        tg, og = c // OG, c % OG
        x_c = xf[tg * T:(tg + 1) * T]                       # [T, D]
        xT = np.ascontiguousarray(
            x_c.T.reshape(XC, KPC, 128, T).transpose(2, 0, 1, 3)
            .reshape(128, XC, KPC * T)).astype(NPBF16)
        w_c = base_w[og * TO:(og + 1) * TO]                 # [TO, D]
        wT = np.ascontiguousarray(
            w_c.reshape(OTN, 128, KT, 128).transpose(3, 0, 2, 1)
            .reshape(128, OTN, KT * 128)).astype(NPBF16)
        bT = np.ascontiguousarray(B_flat[:, og * TO:(og + 1) * TO]).astype(NPBF16)
        bias2 = np.ascontiguousarray(
            base_b[og * TO:(og + 1) * TO].reshape(OTN, 128).T)
        in_maps.append({"xT": xT, "wT": wT, "aT": aT, "gT": gT,
                        "bT": bT, "bias2": bias2, "Rm": Rm})
    return in_maps


def gather_outputs(results):
    """list of 8 per-core result maps -> FULL output [B, S, O]."""
    full = np.empty((B * S, O), dtype=np.float32)
    for c in range(N_CORES):
        tg, og = c // OG, c % OG
        oc = results[c]["out"]                              # [128, OTN, T]
        full[tg * T:(tg + 1) * T, og * TO:(og + 1) * TO] = \
            oc.transpose(2, 1, 0).reshape(T, TO)
    return full.reshape(B, S, O)


_NC_CACHE = {}


def _get_module():
    if "nc" not in _NC_CACHE:
        _NC_CACHE["nc"] = build_module()
    return _NC_CACHE["nc"]


def run_sharded(in_maps, **run_kwargs):
    nc = _get_module()
    return run_bass_kernel_spmd(nc, in_maps, list(range(N_CORES)), **run_kwargs)


def kernel(x, gate_w, base_w, base_b, lora_A, lora_B):
    in_maps = shard_inputs(x, gate_w, base_w, base_b, lora_A, lora_B)
    res = run_sharded(in_maps)
    return gather_outputs(res.results)


# revision 3
# speedup vs baseline: 1.2872x; 1.0696x over previous
"""Trainium2 Bass kernel for a LoRA-MoE layer (gate top-2 softmax routing +
dense base linear + per-expert low-rank adapters), SPMD across 8 NeuronCores.

Math (per token t):
    logits = x @ gate_w.T                      # [E]
    top-2 softmax over logits -> dense w[E] (0 for non-selected)
    out = x @ base_w.T + base_b
        + SCALING * sum_e w[e] * (x @ lora_A[e].T) @ lora_B[e].T

Key identities:
  * w folded into rank-space activations: lora_out = (low * w_rep) @ B_all.T
    with low = x @ A_all.T (A_all: [E*R, D]) -> whole MoE-LoRA is two dense
    matmuls + tiny gating vector math.
  * top-2 softmax via sigmoid: w_e = [l_e >= m2] * sigmoid(2*l_e - m1 - m2)
    (for the top-1 expert this is sigmoid(l1-l2), for top-2 sigmoid(l2-l1)).

Sharding: 8-way data parallel over tokens (T=512 tokens per core), base W
replicated and streamed.  This halves the x-load + phase-A serial head vs a
token x out-feature split; W streaming needs only ~150 GB/s per core.

Performance structure:
  * all matmul operands bf16 (host cast, free) -> PE rate unchanged, HBM
    bytes halved.
  * DMA order: adapters + x chunks first on both rings, W strictly behind x.
  * ~16 dummy matmuls at t~4us keep the PE HAM clock-gate warm before the
    first x chunk lands.
  * single shared 8-slot PSUM pool; out-tile k-loops run back-to-back while
    the gating vector chain (DVE/ACT/GPSIMD) hides behind them; each
    out-tile's B-adapter "stop" matmul is deferred two groups.
"""

import numpy as np
import ml_dtypes

import concourse.bass as bass
import concourse.bass_isa as bass_isa
import concourse.mybir as mybir
import concourse.tile as tile
from concourse import bacc
from concourse.bass_utils import run_bass_kernel_spmd

F32 = mybir.dt.float32
BF16 = mybir.dt.bfloat16
NPBF16 = ml_dtypes.bfloat16

# Problem constants
B, S, D, O = 2, 2048, 4096, 4096
E, R = 8, 16
ER = E * R  # 128
SCALING = 32.0 / 16.0

# Sharding: 8 token groups, W replicated
N_CORES = 8
TG = 8
T = (B * S) // TG       # 512 tokens per core
KT = D // 128           # 32 contraction tiles
OTN = O // 128          # 32 out tiles per core
XC = 8                  # x DMA chunks
KPC = KT // XC          # 4 k-tiles per chunk
NWARM = 16              # PE warm-up matmuls


def build_body(nc, tc, tensors):
    xT, wT, aT, gT, bT, bias2, Rm, out = tensors
    OP = mybir.AluOpType
    ACT = mybir.ActivationFunctionType

    with (
        tc.tile_pool(name="xp", bufs=XC) as xp,
        tc.tile_pool(name="wp", bufs=8) as wp,
        tc.tile_pool(name="cst", bufs=1) as cst,
        tc.tile_pool(name="gw", bufs=1) as gw,
        tc.tile_pool(name="outp", bufs=2) as outp,
        tc.tile_pool(name="ps", bufs=8, space="PSUM") as ps,
    ):
        # ---- DMA program.  sync ring: even x chunks then all W;
        #      scalar ring: a, g, odd x chunks, then smalls. ----
        a_all = cst.tile([128, KT * ER], BF16)
        nc.scalar.dma_start(out=a_all[:], in_=aT[:])
        g_all = cst.tile([128, KT * E], BF16)
        nc.scalar.dma_start(out=g_all[:], in_=gT[:])

        x_tiles = []
        for c in range(XC):
            xc_t = xp.tile([128, KPC * T], BF16, tag="x", name=f"x{c}")
            eng = nc.sync if c % 2 == 0 else nc.scalar
            eng.dma_start(out=xc_t[:], in_=xT[:, c, :])
            x_tiles.append(xc_t)

        bias_sb = cst.tile([128, OTN], F32)
        nc.scalar.dma_start(out=bias_sb[:], in_=bias2[:])
        Rm_sb = cst.tile([E, ER], BF16)
        nc.scalar.dma_start(out=Rm_sb[:], in_=Rm[:])
        bT_sb = cst.tile([ER, O], BF16)
        nc.scalar.dma_start(out=bT_sb[:], in_=bT[:])

        w_tiles = []
        for ot in range(OTN):
            wv = wp.tile([128, KT * 128], BF16, tag="w", name=f"w{ot}")
            nc.sync.dma_start(out=wv[:], in_=wT[:, ot, :])
            w_tiles.append(wv)

        def xs(k):
            """x.T slice [128, T] for k-tile k."""
            return x_tiles[k // KPC][:, (k % KPC) * T:(k % KPC + 1) * T]

        # ---- PE warm-up: dummy matmuls on zeros so the HAM clock gate is
        #      already at 8/8 when the first x chunk lands.  They write the
        #      gate PSUM bank; the real k=0 matmul's start=True wipes them. ----
        warm = cst.tile([128, T], BF16)
        nc.vector.memset(warm[:], 0.0)

        gate_ps = ps.tile([E, T], F32, tag="ps", name="gateps")
        low_ps = ps.tile([ER, T], F32, tag="ps", name="lowps")
        for i in range(NWARM):
            nc.tensor.matmul(gate_ps[:], lhsT=warm[:, :E], rhs=warm[:],
                             start=True, stop=True, skip_group_check=True)

        # ---- phase A: low.T = A_all.T^T @ x.T ; gate.T = g^T @ x.T ----
        for k in range(KT):
            nc.tensor.matmul(gate_ps[:], lhsT=g_all[:, k * E:(k + 1) * E],
                             rhs=xs(k), start=(k == 0), stop=(k == KT - 1),
                             skip_group_check=(k == 0))
            nc.tensor.matmul(low_ps[:], lhsT=a_all[:, k * ER:(k + 1) * ER],
                             rhs=xs(k), start=(k == 0), stop=(k == KT - 1))

        # ---- gating math in [E, t] layout (DVE/ACT/GPSIMD; overlaps the
        #      first base-W matmul groups on the PE) ----
        # w_e = [l_e >= m2] * sigmoid(2*l_e - m1 - m2) * SCALING
        lowT_sb = cst.tile([ER, T], BF16, tag="lowT")
        g_sb = gw.tile([E, T], F32, tag="g")
        nc.scalar.copy(g_sb[:], gate_ps[:])
        m1 = gw.tile([E, T], F32, tag="m1")
        nc.gpsimd.partition_all_reduce(m1[:], g_sb[:], channels=E,
                                       reduce_op=bass_isa.ReduceOp.max)
        eq = gw.tile([E, T], F32, tag="eq")
        nc.vector.tensor_tensor(eq[:], g_sb[:], m1[:], op=OP.is_equal)
        gm = gw.tile([E, T], F32, tag="gm")
        nc.vector.scalar_tensor_tensor(gm[:], in0=eq[:], scalar=-1e30,
                                       in1=g_sb[:], op0=OP.mult, op1=OP.add)
        m2 = gw.tile([E, T], F32, tag="m2")
        nc.gpsimd.partition_all_reduce(m2[:], gm[:], channels=E,
                                       reduce_op=bass_isa.ReduceOp.max)
        t1 = gw.tile([E, T], F32, tag="t1")
        nc.vector.tensor_tensor(t1[:], m1[:], m2[:], op=OP.add)
        s = gw.tile([E, T], F32, tag="s")
        nc.vector.scalar_tensor_tensor(s[:], in0=g_sb[:], scalar=2.0,
                                       in1=t1[:], op0=OP.mult, op1=OP.subtract)
        sig = gw.tile([E, T], F32, tag="sig")
        nc.scalar.activation(sig[:], s[:], ACT.Sigmoid)
        mask = gw.tile([E, T], F32, tag="mask")
        nc.vector.tensor_tensor(mask[:], g_sb[:], m2[:], op=OP.is_ge)
        wsc = gw.tile([E, T], BF16, tag="wsc")
        nc.vector.scalar_tensor_tensor(wsc[:], in0=sig[:], scalar=SCALING,
                                       in1=mask[:], op0=OP.mult, op1=OP.mult)

        # ---- phase B: out.T[ot] = sum_k W[ot,k]^T @ x.T (+ B^T @ low_w.T) ----
        def w_group(ot, pb):
            for k in range(KT):
                nc.tensor.matmul(pb[:], lhsT=w_tiles[ot][:, k * 128:(k + 1) * 128],
                                 rhs=xs(k), start=(k == 0), stop=False)

        def stop_group(ot, pb):
            nc.tensor.matmul(pb[:], lhsT=bT_sb[:, ot * 128:(ot + 1) * 128],
                             rhs=lowT_sb[:], start=False, stop=True)
            o_sb = outp.tile([128, T], F32, tag="o", name=f"o{ot}")
            nc.vector.tensor_scalar(o_sb[:], pb[:],
                                    scalar1=bias_sb[:, ot:ot + 1],
                                    scalar2=None, op0=OP.add)
            nc.gpsimd.dma_start(out=out[:, ot, :], in_=o_sb[:])

        pbs = []
        for ot in range(OTN):
            pb = ps.tile([128, T], F32, tag="ps", name=f"pb{ot}")
            pbs.append(pb)
            w_group(ot, pb)
            if ot == 1:
                # replicate each expert weight over its 16 ranks via a tiny
                # matmul, then fold into the rank-space activations.  Emitted
                # after ot1's W MMs so the PE never waits on the gating chain.
                wrep_ps = ps.tile([ER, T], F32, tag="ps", name="wrep")
                nc.tensor.matmul(wrep_ps[:], lhsT=Rm_sb[:], rhs=wsc[:],
                                 start=True, stop=True)
                wrep_sb = gw.tile([ER, T], F32, tag="wrepsb")
                nc.scalar.copy(wrep_sb[:], wrep_ps[:])
                # low_w.T = low.T * w_rep (DVE: one PSUM operand only)
                nc.vector.tensor_tensor(lowT_sb[:], low_ps[:], wrep_sb[:],
                                        op=OP.mult)
            if ot >= 2:
                stop_group(ot - 2, pbs[ot - 2])
        stop_group(OTN - 2, pbs[OTN - 2])
        stop_group(OTN - 1, pbs[OTN - 1])


def build_module(debug=False):
    nc = bacc.Bacc("TRN2", target_bir_lowering=False, debug=debug)
    xT = nc.dram_tensor("xT", [128, XC, KPC * T], BF16, kind="ExternalInput")
    wT = nc.dram_tensor("wT", [128, OTN, KT * 128], BF16, kind="ExternalInput")
    aT = nc.dram_tensor("aT", [128, KT * ER], BF16, kind="ExternalInput")
    gT = nc.dram_tensor("gT", [128, KT * E], BF16, kind="ExternalInput")
    bT = nc.dram_tensor("bT", [ER, O], BF16, kind="ExternalInput")
    bias2 = nc.dram_tensor("bias2", [128, OTN], F32, kind="ExternalInput")
    Rm = nc.dram_tensor("Rm", [E, ER], BF16, kind="ExternalInput")
    out = nc.dram_tensor("out", [128, OTN, T], F32, kind="ExternalOutput")
    with tile.TileContext(nc) as tc:
        build_body(nc, tc, (xT, wT, aT, gT, bT, bias2, Rm, out))
    nc.compile()
    return nc


def shard_inputs(x, gate_w, base_w, base_b, lora_A, lora_B):
    """FULL inputs -> list of 8 per-core input maps (host-side, free)."""
    x = np.asarray(x, dtype=np.float32)
    gate_w = np.asarray(gate_w, dtype=np.float32)
    base_w = np.asarray(base_w, dtype=np.float32)
    base_b = np.asarray(base_b, dtype=np.float32)
    lora_A = np.asarray(lora_A, dtype=np.float32)
    lora_B = np.asarray(lora_B, dtype=np.float32)

    xf = x.reshape(B * S, D)
    # replicated tensors
    gT = np.ascontiguousarray(
        gate_w.T.reshape(KT, 128, E).transpose(1, 0, 2)
        .reshape(128, KT * E)).astype(NPBF16)
    A_flat = lora_A.reshape(ER, D)
    aT = np.ascontiguousarray(
        A_flat.T.reshape(KT, 128, ER).transpose(1, 0, 2)
        .reshape(128, KT * ER)).astype(NPBF16)
    B_flat = lora_B.transpose(0, 2, 1).reshape(ER, O)   # [er, o]
    bT = np.ascontiguousarray(B_flat).astype(NPBF16)
    Rm = np.repeat(np.eye(E, dtype=np.float32), R, axis=1).astype(NPBF16)
    wT = np.ascontiguousarray(
        base_w.reshape(OTN, 128, KT, 128).transpose(3, 0, 2, 1)
        .reshape(128, OTN, KT * 128)).astype(NPBF16)
    bias2 = np.ascontiguousarray(base_b.reshape(OTN, 128).T)

    in_maps = []
    for c in range(N_CORES):
        x_c = xf[c * T:(c + 1) * T]                         # [T, D]
        xT = np.ascontiguousarray(
            x_c.T.reshape(XC, KPC, 128, T).transpose(2, 0, 1, 3)
            .reshape(128, XC, KPC * T)).astype(NPBF16)
        in_maps.append({"xT": xT, "wT": wT, "aT": aT, "gT": gT,
                        "bT": bT, "bias2": bias2, "Rm": Rm})
    return in_maps


def gather_outputs(results):
    """list of 8 per-core result maps -> FULL output [B, S, O]."""
    full = np.empty((B * S, O), dtype=np.float32)
    for c in range(N_CORES):
        oc = results[c]["out"]                              # [128, OTN, T]
        full[c * T:(c + 1) * T, :] = oc.transpose(2, 1, 0).reshape(T, O)
    return full.reshape(B, S, O)


_NC_CACHE = {}


def _get_module():
    if "nc" not in _NC_CACHE:
        _NC_CACHE["nc"] = build_module()
    return _NC_CACHE["nc"]


def run_sharded(in_maps, **run_kwargs):
    nc = _get_module()
    return run_bass_kernel_spmd(nc, in_maps, list(range(N_CORES)), **run_kwargs)


def kernel(x, gate_w, base_w, base_b, lora_A, lora_B):
    in_maps = shard_inputs(x, gate_w, base_w, base_b, lora_A, lora_B)
    res = run_sharded(in_maps)
    return gather_outputs(res.results)


# revision 5
# speedup vs baseline: 1.3100x; 1.0177x over previous
"""Trainium2 Bass kernel for a LoRA-MoE layer (gate top-2 softmax routing +
dense base linear + per-expert low-rank adapters), SPMD across 8 NeuronCores.

Math (per token t):
    logits = x @ gate_w.T                      # [E]
    top-2 softmax over logits -> dense w[E] (0 for non-selected)
    out = x @ base_w.T + base_b
        + SCALING * sum_e w[e] * (x @ lora_A[e].T) @ lora_B[e].T

Key identities:
  * w folded into rank-space activations: lora_out = (low * w_rep) @ B_all.T
    with low = x @ A_all.T (A_all: [E*R, D]) -> whole MoE-LoRA is two dense
    matmuls + tiny gating vector math.
  * top-2 softmax via sigmoid: w_e = [l_e >= m2] * sigmoid(2*l_e - m1 - m2)
    (for the top-1 expert this is sigmoid(l1-l2), for top-2 sigmoid(l2-l1)).

Sharding: 8-way data parallel over tokens (T=512 tokens per core), base W
replicated and streamed.  This halves the x-load + phase-A serial head vs a
token x out-feature split; W streaming needs only ~150 GB/s per core.

Performance structure:
  * all matmul operands bf16 (host cast, free) -> PE rate unchanged, HBM
    bytes halved.
  * DMA order: adapters + x chunks first on both rings, W strictly behind x.
  * ~16 dummy matmuls at t~4us keep the PE HAM clock-gate warm before the
    first x chunk lands.
  * single shared 8-slot PSUM pool; out-tile k-loops run back-to-back while
    the gating vector chain (DVE/ACT/GPSIMD) hides behind them; each
    out-tile's B-adapter "stop" matmul is deferred two groups.
"""

import numpy as np
import ml_dtypes

import concourse.bass as bass
import concourse.bass_isa as bass_isa
import concourse.mybir as mybir
import concourse.tile as tile
from concourse import bacc
from concourse.bass_utils import run_bass_kernel_spmd

F32 = mybir.dt.float32
BF16 = mybir.dt.bfloat16
NPBF16 = ml_dtypes.bfloat16

# Problem constants
B, S, D, O = 2, 2048, 4096, 4096
E, R = 8, 16
ER = E * R  # 128
SCALING = 32.0 / 16.0

# Sharding: 8 token groups, W replicated
N_CORES = 8
TG = 8
T = (B * S) // TG       # 512 tokens per core
KT = D // 128           # 32 contraction tiles
OTN = O // 128          # 32 out tiles per core
XC = 8                  # x DMA chunks
KPC = KT // XC          # 4 k-tiles per chunk
NWARM = 8              # PE warm-up matmuls


def build_body(nc, tc, tensors):
    xT, wT, aT, gT, bT, bias2, Rm, out = tensors
    OP = mybir.AluOpType
    ACT = mybir.ActivationFunctionType

    with (
        tc.tile_pool(name="xp", bufs=XC) as xp,
        tc.tile_pool(name="wp", bufs=8) as wp,
        tc.tile_pool(name="cst", bufs=1) as cst,
        tc.tile_pool(name="gw", bufs=1) as gw,
        tc.tile_pool(name="outp", bufs=2) as outp,
        tc.tile_pool(name="ps", bufs=8, space="PSUM") as ps,
    ):
        # ---- DMA program.  Criticality order: g first (tiny), then a/x
        #      chunks interleaved across both rings so phase A streams as
        #      soon as each chunk lands; W strictly behind x on the sync
        #      ring; bias/Rm/bT (needed ~35us+) behind x on the scalar ring. ----
        g_all = cst.tile([128, KT * E], BF16)
        nc.scalar.dma_start(out=g_all[:], in_=gT[:])

        a_all = cst.tile([128, KT * ER], BF16)
        x_tiles = []
        APC = KPC * ER  # a elements per chunk (per partition)
        for c in range(XC):
            eng = nc.sync if c % 2 == 0 else nc.scalar
            eng.dma_start(out=a_all[:, c * APC:(c + 1) * APC],
                          in_=aT[:, c * APC:(c + 1) * APC])
            xc_t = xp.tile([128, KPC * T], BF16, tag="x", name=f"x{c}")
            eng.dma_start(out=xc_t[:], in_=xT[:, c, :])
            x_tiles.append(xc_t)

        bias_sb = cst.tile([128, OTN], F32)
        nc.scalar.dma_start(out=bias_sb[:], in_=bias2[:])
        Rm_sb = cst.tile([E, ER], BF16)
        nc.scalar.dma_start(out=Rm_sb[:], in_=Rm[:])
        bT_sb = cst.tile([ER, O], BF16)
        nc.scalar.dma_start(out=bT_sb[:], in_=bT[:])

        w_tiles = []
        for ot in range(OTN):
            wv = wp.tile([128, KT * 128], BF16, tag="w", name=f"w{ot}")
            nc.sync.dma_start(out=wv[:], in_=wT[:, ot, :])
            w_tiles.append(wv)

        def xs(k):
            """x.T slice [128, T] for k-tile k."""
            return x_tiles[k // KPC][:, (k % KPC) * T:(k % KPC + 1) * T]

        # ---- PE warm-up: dummy matmuls on zeros so the HAM clock gate is
        #      already at 8/8 when the first x chunk lands.  They write the
        #      gate PSUM bank; the real k=0 matmul's start=True wipes them. ----
        warm = cst.tile([128, T], BF16)
        nc.vector.memset(warm[:], 0.0)

        gate_ps = ps.tile([E, T], F32, tag="ps", name="gateps")
        low_ps = ps.tile([ER, T], F32, tag="ps", name="lowps")
        for i in range(NWARM):
            nc.tensor.matmul(gate_ps[:], lhsT=warm[:, :E], rhs=warm[:],
                             start=True, stop=True, skip_group_check=True)

        # ---- phase A: low.T = A_all.T^T @ x.T ; gate.T = g^T @ x.T ----
        for k in range(KT):
            nc.tensor.matmul(gate_ps[:], lhsT=g_all[:, k * E:(k + 1) * E],
                             rhs=xs(k), start=(k == 0), stop=(k == KT - 1),
                             skip_group_check=(k == 0))
            nc.tensor.matmul(low_ps[:], lhsT=a_all[:, k * ER:(k + 1) * ER],
                             rhs=xs(k), start=(k == 0), stop=(k == KT - 1))

        # ---- gating math in [E, t] layout (DVE/ACT/GPSIMD; overlaps the
        #      first base-W matmul groups on the PE) ----
        # w_e = [l_e >= m2] * sigmoid(2*l_e - m1 - m2) * SCALING
        lowT_sb = cst.tile([ER, T], BF16, tag="lowT")
        g_sb = gw.tile([E, T], F32, tag="g")
        nc.scalar.copy(g_sb[:], gate_ps[:])
        m1 = gw.tile([E, T], F32, tag="m1")
        nc.gpsimd.partition_all_reduce(m1[:], g_sb[:], channels=E,
                                       reduce_op=bass_isa.ReduceOp.max)
        eq = gw.tile([E, T], F32, tag="eq")
        nc.vector.tensor_tensor(eq[:], g_sb[:], m1[:], op=OP.is_equal)
        gm = gw.tile([E, T], F32, tag="gm")
        nc.vector.scalar_tensor_tensor(gm[:], in0=eq[:], scalar=-1e30,
                                       in1=g_sb[:], op0=OP.mult, op1=OP.add)
        m2 = gw.tile([E, T], F32, tag="m2")
        nc.gpsimd.partition_all_reduce(m2[:], gm[:], channels=E,
                                       reduce_op=bass_isa.ReduceOp.max)
        t1 = gw.tile([E, T], F32, tag="t1")
        nc.vector.tensor_tensor(t1[:], m1[:], m2[:], op=OP.add)
        s = gw.tile([E, T], F32, tag="s")
        nc.vector.scalar_tensor_tensor(s[:], in0=g_sb[:], scalar=2.0,
                                       in1=t1[:], op0=OP.mult, op1=OP.subtract)
        sig = gw.tile([E, T], F32, tag="sig")
        nc.scalar.activation(sig[:], s[:], ACT.Sigmoid)
        mask = gw.tile([E, T], F32, tag="mask")
        nc.vector.tensor_tensor(mask[:], g_sb[:], m2[:], op=OP.is_ge)
        wsc = gw.tile([E, T], BF16, tag="wsc")
        nc.vector.scalar_tensor_tensor(wsc[:], in0=sig[:], scalar=SCALING,
                                       in1=mask[:], op0=OP.mult, op1=OP.mult)

        # ---- phase B: out.T[ot] = sum_k W[ot,k]^T @ x.T (+ B^T @ low_w.T) ----
        def w_group(ot, pb):
            for k in range(KT):
                nc.tensor.matmul(pb[:], lhsT=w_tiles[ot][:, k * 128:(k + 1) * 128],
                                 rhs=xs(k), start=(k == 0), stop=False)

        def stop_group(ot, pb):
            nc.tensor.matmul(pb[:], lhsT=bT_sb[:, ot * 128:(ot + 1) * 128],
                             rhs=lowT_sb[:], start=False, stop=True)
            o_sb = outp.tile([128, T], BF16, tag="o", name=f"o{ot}")
            nc.vector.tensor_scalar(o_sb[:], pb[:],
                                    scalar1=bias_sb[:, ot:ot + 1],
                                    scalar2=None, op0=OP.add)
            nc.gpsimd.dma_start(out=out[:, ot, :], in_=o_sb[:])

        pbs = []
        for ot in range(OTN):
            pb = ps.tile([128, T], F32, tag="ps", name=f"pb{ot}")
            pbs.append(pb)
            w_group(ot, pb)
            if ot == 1:
                # replicate each expert weight over its 16 ranks via a tiny
                # matmul, then fold into the rank-space activations.  Emitted
                # after ot1's W MMs so the PE never waits on the gating chain.
                wrep_ps = ps.tile([ER, T], F32, tag="ps", name="wrep")
                nc.tensor.matmul(wrep_ps[:], lhsT=Rm_sb[:], rhs=wsc[:],
                                 start=True, stop=True)
                wrep_sb = gw.tile([ER, T], F32, tag="wrepsb")
                nc.scalar.copy(wrep_sb[:], wrep_ps[:])
                # low_w.T = low.T * w_rep (DVE: one PSUM operand only)
                nc.vector.tensor_tensor(lowT_sb[:], low_ps[:], wrep_sb[:],
                                        op=OP.mult)
            if ot >= 2:
                stop_group(ot - 2, pbs[ot - 2])
        stop_group(OTN - 2, pbs[OTN - 2])
        stop_group(OTN - 1, pbs[OTN - 1])


def build_module(debug=False):
    nc = bacc.Bacc("TRN2", target_bir_lowering=False, debug=debug)
    xT = nc.dram_tensor("xT", [128, XC, KPC * T], BF16, kind="ExternalInput")
    wT = nc.dram_tensor("wT", [128, OTN, KT * 128], BF16, kind="ExternalInput")
    aT = nc.dram_tensor("aT", [128, KT * ER], BF16, kind="ExternalInput")
    gT = nc.dram_tensor("gT", [128, KT * E], BF16, kind="ExternalInput")
    bT = nc.dram_tensor("bT", [ER, O], BF16, kind="ExternalInput")
    bias2 = nc.dram_tensor("bias2", [128, OTN], F32, kind="ExternalInput")
    Rm = nc.dram_tensor("Rm", [E, ER], BF16, kind="ExternalInput")
    out = nc.dram_tensor("out", [128, OTN, T], BF16, kind="ExternalOutput")
    with tile.TileContext(nc) as tc:
        build_body(nc, tc, (xT, wT, aT, gT, bT, bias2, Rm, out))
    nc.compile()
    return nc


def shard_inputs(x, gate_w, base_w, base_b, lora_A, lora_B):
    """FULL inputs -> list of 8 per-core input maps (host-side, free)."""
    x = np.asarray(x, dtype=np.float32)
    gate_w = np.asarray(gate_w, dtype=np.float32)
    base_w = np.asarray(base_w, dtype=np.float32)
    base_b = np.asarray(base_b, dtype=np.float32)
    lora_A = np.asarray(lora_A, dtype=np.float32)
    lora_B = np.asarray(lora_B, dtype=np.float32)

    xf = x.reshape(B * S, D)
    # replicated tensors
    gT = np.ascontiguousarray(
        gate_w.T.reshape(KT, 128, E).transpose(1, 0, 2)
        .reshape(128, KT * E)).astype(NPBF16)
    A_flat = lora_A.reshape(ER, D)
    aT = np.ascontiguousarray(
        A_flat.T.reshape(KT, 128, ER).transpose(1, 0, 2)
        .reshape(128, KT * ER)).astype(NPBF16)
    B_flat = lora_B.transpose(0, 2, 1).reshape(ER, O)   # [er, o]
    bT = np.ascontiguousarray(B_flat).astype(NPBF16)
    Rm = np.repeat(np.eye(E, dtype=np.float32), R, axis=1).astype(NPBF16)
    wT = np.ascontiguousarray(
        base_w.reshape(OTN, 128, KT, 128).transpose(3, 0, 2, 1)
        .reshape(128, OTN, KT * 128)).astype(NPBF16)
    bias2 = np.ascontiguousarray(base_b.reshape(OTN, 128).T)

    in_maps = []
    for c in range(N_CORES):
        x_c = xf[c * T:(c + 1) * T]                         # [T, D]
        xT = np.ascontiguousarray(
            x_c.T.reshape(XC, KPC, 128, T).transpose(2, 0, 1, 3)
            .reshape(128, XC, KPC * T)).astype(NPBF16)
        in_maps.append({"xT": xT, "wT": wT, "aT": aT, "gT": gT,
                        "bT": bT, "bias2": bias2, "Rm": Rm})
    return in_maps


def gather_outputs(results):
    """list of 8 per-core result maps -> FULL output [B, S, O]."""
    full = np.empty((B * S, O), dtype=np.float32)
    for c in range(N_CORES):
        oc = np.asarray(results[c]["out"], dtype=np.float32)  # [128, OTN, T]
        full[c * T:(c + 1) * T, :] = oc.transpose(2, 1, 0).reshape(T, O)
    return full.reshape(B, S, O)


_NC_CACHE = {}


def _get_module():
    if "nc" not in _NC_CACHE:
        _NC_CACHE["nc"] = build_module()
    return _NC_CACHE["nc"]


def run_sharded(in_maps, **run_kwargs):
    nc = _get_module()
    return run_bass_kernel_spmd(nc, in_maps, list(range(N_CORES)), **run_kwargs)


def kernel(x, gate_w, base_w, base_b, lora_A, lora_B):
    in_maps = shard_inputs(x, gate_w, base_w, base_b, lora_A, lora_B)
    res = run_sharded(in_maps)
    return gather_outputs(res.results)
